# revision 1
# baseline (speedup 1.0000x reference)
"""Trainium2 Bass kernel for ContrastiveMaskedPatchSimilarity loss.

Computes: per-position cosine similarity along the channel axis of two
[32, 256, 64, 64] f32 tensors, then a masked mean -> scalar.

Strategy (pure data parallel over 8 NeuronCores, batch-sharded 4 each):
  - Layout on chip: [channel-chunk (128) = partitions, spatial (4096) = free].
    DMA of u/m tiles is perfectly contiguous per partition (16KB rows).
  - Elementwise products (u*m, u*u, m*m) on DVE/ACT, written as bf16.
  - Channel reduction via TensorE: product slice [128ch x 128pos] is the
    *stationary* operand (lhsT), rhs = ones[128,1] bf16 -> out[128pos, 1]
    lands position-major in PSUM, so the epilogue runs with all 128
    partitions busy.
  - Epilogue per batch: num/(sqrt(uu*mm)), fused multiply+reduce with the
    (host-pretransposed) mask, free-axis reduction -> [128, 8] partials.
  - Host: sum partials over cores, divide.
"""

import sys
from contextlib import ExitStack

import numpy as np

sys.path.insert(0, "/opt/trn_rl_repo")

import ml_dtypes  # noqa: E402

import concourse.bass as bass  # noqa: E402
import concourse.tile as tile  # noqa: E402
from concourse import bacc, mybir  # noqa: E402
from concourse.bass_utils import run_bass_kernel_spmd  # noqa: E402

B, C, H, W = 32, 256, 64, 64
NCORES = 8
BL = B // NCORES  # batches per core: 4
HWX = H * W  # 4096
ROWS = BL * C  # 1024
NPB = HWX // 128  # position blocks per batch: 32
NCHUNK = C // 128  # channel chunks: 2

F32 = mybir.dt.float32
BF16 = mybir.dt.bfloat16

_CACHED_NC = None


def build_nc():
    nc = bacc.Bacc(
        "TRN2", target_bir_lowering=False, debug=False, num_devices=NCORES
    )
    u_d = nc.dram_tensor("u", [ROWS, HWX], F32, kind="ExternalInput")
    m_d = nc.dram_tensor("m", [ROWS, HWX], F32, kind="ExternalInput")
    # mask, pre-transposed on host to [p_in (128), b*NPB + pb (128)] f32
    mk_d = nc.dram_tensor("maskf", [128, BL * NPB], F32, kind="ExternalInput")
    ones_d = nc.dram_tensor("ones", [128, 1], BF16, kind="ExternalInput")
    # out[:, 0:BL] = per-batch sum(sim*mask) partials (per partition)
    # out[:, BL:2BL] = per-batch sum(mask) partials (per partition)
    out_d = nc.dram_tensor("out", [128, 2 * BL], F32, kind="ExternalOutput")

    with tile.TileContext(nc) as tc, ExitStack() as ctx:
        const_pool = ctx.enter_context(tc.tile_pool(name="const", bufs=1))
        in_pool = ctx.enter_context(tc.tile_pool(name="inp", bufs=6))
        tmp_pool = ctx.enter_context(tc.tile_pool(name="tmp", bufs=3))
        ep_pool = ctx.enter_context(tc.tile_pool(name="ep", bufs=2))
        acc_pool = ctx.enter_context(tc.tile_pool(name="acc", bufs=1))
        psum_pool = ctx.enter_context(
            tc.tile_pool(name="psum", bufs=2, space="PSUM")
        )

        ones_t = const_pool.tile([128, 1], BF16)
        nc.sync.dma_start(ones_t[:], ones_d[:, :])
        maskf_t = const_pool.tile([128, BL * NPB], F32)
        nc.sync.dma_start(maskf_t[:], mk_d[:, :])
        acc_t = acc_pool.tile([128, 2 * BL], F32)
        # mask-only sums don't depend on tensor data: do them up front
        for b in range(BL):
            nc.vector.tensor_reduce(
                acc_t[:, BL + b : BL + b + 1],
                maskf_t[:, b * NPB : (b + 1) * NPB],
                axis=mybir.AxisListType.X,
                op=mybir.AluOpType.add,
            )

        HHX = HWX // 2  # half-tile free dim (1MB DMAs, earlier pipeline ramp)
        HPB = HHX // 128  # position blocks per half: 16
        mm_ctr = 0
        for b in range(BL):
            # PSUM cols: ch*3*NPB + stat*NPB + (h*HPB + pb)
            P = psum_pool.tile([128, NCHUNK * 3 * NPB], F32)
            for ch in range(NCHUNK):
                row0 = b * C + ch * 128
                for h in range(2):
                    csl = slice(h * HHX, (h + 1) * HHX)
                    u_t = in_pool.tile([128, HHX], F32, tag="u")
                    nc.sync.dma_start(u_t[:], u_d[row0 : row0 + 128, csl])
                    m_t = in_pool.tile([128, HHX], F32, tag="m")
                    nc.gpsimd.dma_start(m_t[:], m_d[row0 : row0 + 128, csl])

                    num_t = tmp_pool.tile([128, HHX], BF16, tag="num")
                    nc.vector.tensor_mul(num_t[:], u_t[:], m_t[:])
                    uu_t = tmp_pool.tile([128, HHX], BF16, tag="uu")
                    nc.scalar.square(uu_t[:], u_t[:])
                    mm_t = tmp_pool.tile([128, HHX], BF16, tag="mm")
                    # balance m*m between DVE (faster) and ACT so neither
                    # engine exceeds the DMA roofline
                    if mm_ctr % 3 == 0:
                        nc.vector.tensor_mul(mm_t[:], m_t[:], m_t[:])
                    else:
                        nc.scalar.square(mm_t[:], m_t[:])
                    mm_ctr += 1

                    for s, t in enumerate((num_t, uu_t, mm_t)):
                        base = ch * 3 * NPB + s * NPB + h * HPB
                        for pb in range(HPB):
                            nc.tensor.matmul(
                                P[:, base + pb : base + pb + 1],
                                t[:, pb * 128 : (pb + 1) * 128],
                                ones_t[:, :],
                                start=True,
                                stop=True,
                            )

            # epilogue for batch b (position-major [128, NPB] tiles)
            def psl(ch, s):
                c0 = ch * 3 * NPB + s * NPB
                return P[:, c0 : c0 + NPB]

            # DVE has a single PSUM read port: copy chunk-0 stats to SBUF
            # on ACT first, then add with only one PSUM operand per op.
            n0 = ep_pool.tile([128, NPB], F32, tag="n0")
            nc.scalar.copy(n0[:], psl(0, 0))
            u0 = ep_pool.tile([128, NPB], F32, tag="u0")
            nc.scalar.copy(u0[:], psl(0, 1))
            m0 = ep_pool.tile([128, NPB], F32, tag="m0")
            nc.scalar.copy(m0[:], psl(0, 2))
            numv = ep_pool.tile([128, NPB], F32, tag="numv")
            nc.vector.tensor_add(numv[:], n0[:], psl(1, 0))
            uuv = ep_pool.tile([128, NPB], F32, tag="uuv")
            nc.vector.tensor_add(uuv[:], u0[:], psl(1, 1))
            mmv = ep_pool.tile([128, NPB], F32, tag="mmv")
            nc.vector.tensor_add(mmv[:], m0[:], psl(1, 2))
            d2 = ep_pool.tile([128, NPB], F32, tag="d2")
            nc.vector.tensor_mul(d2[:], uuv[:], mmv[:])
            r = ep_pool.tile([128, NPB], F32, tag="r")
            nc.vector.reciprocal(r[:], d2[:])
            rs = ep_pool.tile([128, NPB], F32, tag="rs")
            nc.scalar.sqrt(rs[:], r[:])
            sim_t = ep_pool.tile([128, NPB], F32, tag="sim")
            nc.vector.tensor_mul(sim_t[:], numv[:], rs[:])
            simmask = ep_pool.tile([128, NPB], F32, tag="simmask")
            nc.vector.tensor_mul(
                simmask[:], sim_t[:], maskf_t[:, b * NPB : (b + 1) * NPB]
            )
            nc.vector.tensor_reduce(
                acc_t[:, b : b + 1],
                simmask[:],
                axis=mybir.AxisListType.X,
                op=mybir.AluOpType.add,
            )

        nc.sync.dma_start(out_d[:, :], acc_t[:])

    nc.compile()
    return nc


def get_nc():
    global _CACHED_NC
    if _CACHED_NC is None:
        _CACHED_NC = build_nc()
    return _CACHED_NC


def make_in_maps(unmasked, masked, latent_mask):
    ones = np.ones((128, 1), dtype=ml_dtypes.bfloat16)
    in_maps = []
    for i in range(NCORES):
        sl = slice(i * BL, (i + 1) * BL)
        u = np.ascontiguousarray(unmasked[sl]).reshape(ROWS, HWX)
        m = np.ascontiguousarray(masked[sl]).reshape(ROWS, HWX)
        mk = (
            latent_mask[sl]
            .reshape(128, 128)
            .T.astype(np.float32)
        )
        in_maps.append(
            {
                "u": u,
                "m": m,
                "maskf": np.ascontiguousarray(mk),
                "ones": ones,
            }
        )
    return in_maps


def _finalize(results):
    num = 0.0
    den = 0.0
    for res in results:
        out = np.asarray(res["out"], dtype=np.float64)
        num += out[:, :BL].sum()
        den += out[:, BL:].sum()
    return np.float32(num / den)


def kernel(unmasked_latent_tensors, masked_latent_tensors, latent_mask, **kw):
    nc = get_nc()
    in_maps = make_in_maps(
        np.asarray(unmasked_latent_tensors, dtype=np.float32),
        np.asarray(masked_latent_tensors, dtype=np.float32),
        np.asarray(latent_mask),
    )
    res = run_bass_kernel_spmd(nc, in_maps, list(range(NCORES)))
    return _finalize(res.results)


def kernel_traced(unmasked_latent_tensors, masked_latent_tensors, latent_mask):
    """Like kernel() but with NTFF tracing; returns (value, BassKernelResults)."""
    nc = get_nc()
    in_maps = make_in_maps(
        np.asarray(unmasked_latent_tensors, dtype=np.float32),
        np.asarray(masked_latent_tensors, dtype=np.float32),
        np.asarray(latent_mask),
    )
    res = run_bass_kernel_spmd(nc, in_maps, list(range(NCORES)), trace=True)
    return _finalize(res.results), res



# revision 11
# speedup vs baseline: 1.1017x; 1.1017x over previous
"""Trainium2 Bass kernel for ContrastiveMaskedPatchSimilarity loss.

Computes: per-position cosine similarity along the channel axis of two
[32, 256, 64, 64] f32 tensors, then a masked mean -> scalar.

Strategy (pure data parallel over 8 NeuronCores, batch-sharded 4 each):
  - Layout on chip: [channel-chunk (128) = partitions, spatial (4096) = free].
    DMA of u/m tiles is perfectly contiguous per partition (8KB rows).
  - Input DMAs are spread across all three DMA-issuing engines (sync/SP,
    scalar/ACT hardware DGE + gpsimd software DGE) with a weighted
    round-robin, so the 16 DMA engines see ~3 queues each and can reach
    the 360 GB/s per-core HBM roofline instead of ~290 with 2 queues.
  - Elementwise products (u*m, u*u, m*m) on DVE/ACT/gpsimd, written bf16.
  - Channel reduction via TensorE: product slice [128ch x 128pos] is the
    stationary operand (lhsT), rhs = ones[128,1] bf16 -> out[128pos, 1]
    lands position-major in PSUM. The two channel chunks accumulate into
    the SAME PSUM column (start on chunk 0, stop on chunk 1), so no
    cross-chunk adds are needed in the epilogue.
  - Epilogue per batch: d2 = uu*mm, rsqrt via reciprocal+sqrt, fold the
    mask into the rsqrt factor, then one fused multiply+reduce
    (tensor_tensor_reduce) of num * (rsqrt*mask) -> acc[:, b].
  - Host: sum sim partials over cores; mask denominator directly on host.
"""

import os
import sys
from contextlib import ExitStack

import numpy as np

# bisect flags (default = full optimization set); not read by the harness
_F_SCALAR_DMA = os.environ.get("K_SCALAR_DMA", "1") == "1"
_F_GPSIMD_MM = os.environ.get("K_GPSIMD_MM", "1") == "1"
_F_PSUM_ACC = os.environ.get("K_PSUM_ACC", "1") == "1"
# tensor_tensor_reduce faults at NEFF runtime on this stack (bisected
# 2026-08-08); keep the unfused mul+reduce pair unless explicitly enabled
_F_TTR = os.environ.get("K_TTR", "0") == "1"

sys.path.insert(0, "/opt/trn_rl_repo")

import ml_dtypes  # noqa: E402

import concourse.bass as bass  # noqa: E402
import concourse.tile as tile  # noqa: E402
from concourse import bacc, mybir  # noqa: E402
from concourse.bass_utils import run_bass_kernel_spmd  # noqa: E402

B, C, H, W = 32, 256, 64, 64
NCORES = 8
BL = B // NCORES  # batches per core: 4
HWX = H * W  # 4096
ROWS = BL * C  # 1024
NPB = HWX // 128  # position blocks per batch: 32
NCHUNK = C // 128  # channel chunks: 2

F32 = mybir.dt.float32
BF16 = mybir.dt.bfloat16

_CACHED_NC = None


def build_nc():
    nc = bacc.Bacc(
        "TRN2", target_bir_lowering=False, debug=False, num_devices=NCORES
    )
    u_d = nc.dram_tensor("u", [ROWS, HWX], F32, kind="ExternalInput")
    m_d = nc.dram_tensor("m", [ROWS, HWX], F32, kind="ExternalInput")
    # mask, pre-transposed on host to [p_in (128), b*NPB + pb (128)] f32
    mk_d = nc.dram_tensor("maskf", [128, BL * NPB], F32, kind="ExternalInput")
    ones_d = nc.dram_tensor("ones", [128, 1], BF16, kind="ExternalInput")
    # out[:, b] = per-batch sum(sim*mask) partials (per partition)
    out_d = nc.dram_tensor("out", [128, BL], F32, kind="ExternalOutput")

    # weighted round-robin across the DMA-issuing engines; gpsimd's
    # software DGE sustains slightly more per-queue bandwidth than the
    # hardware queues, so it gets a larger share.
    dma_engines = None  # filled inside context
    if _F_SCALAR_DMA:
        shares = [0.3125, 0.3125, 0.375]  # sync, scalar, gpsimd
    else:
        shares = [0.465, 0.535]  # sync, gpsimd
    deficit = [0.0] * len(shares)

    def next_queue():
        i = max(range(len(shares)), key=lambda j: deficit[j])
        deficit[i] -= 1.0
        for j in range(len(shares)):
            deficit[j] += shares[j]
        return dma_engines[i]

    with tile.TileContext(nc) as tc, ExitStack() as ctx:
        if _F_SCALAR_DMA:
            dma_engines = (nc.sync, nc.scalar, nc.gpsimd)
        else:
            dma_engines = (nc.sync, nc.gpsimd)
        const_pool = ctx.enter_context(tc.tile_pool(name="const", bufs=1))
        in_pool = ctx.enter_context(tc.tile_pool(name="inp", bufs=7))
        tmp_pool = ctx.enter_context(tc.tile_pool(name="tmp", bufs=3))
        ep_pool = ctx.enter_context(tc.tile_pool(name="ep", bufs=2))
        acc_pool = ctx.enter_context(tc.tile_pool(name="acc", bufs=1))
        psum_pool = ctx.enter_context(
            tc.tile_pool(name="psum", bufs=3, space="PSUM")
        )

        HHX = HWX // 2  # half-tile free dim (1MB DMAs)
        HPB = HHX // 128  # position blocks per half: 16

        # emitted lazily, after the first tile's input DMAs, so the big
        # streams start flowing as early as possible
        ones_t = None
        maskf_t = None
        acc_t = acc_pool.tile([128, BL], F32)

        k = 0  # global half-tile counter
        for b in range(BL):
            # PSUM cols: s*NPB + (h*HPB + pb); chunks accumulate in place
            # (or, with _F_PSUM_ACC off, land in per-chunk columns that the
            # epilogue adds)
            ncols = 3 * NPB if _F_PSUM_ACC else NCHUNK * 3 * NPB
            P = psum_pool.tile([128, ncols], F32)
            for ch in range(NCHUNK):
                row0 = b * C + ch * 128
                for h in range(2):
                    csl = slice(h * HHX, (h + 1) * HHX)
                    u_t = in_pool.tile([128, HHX], F32, tag="u")
                    next_queue().dma_start(u_t[:], u_d[row0 : row0 + 128, csl])
                    m_t = in_pool.tile([128, HHX], F32, tag="m")
                    next_queue().dma_start(m_t[:], m_d[row0 : row0 + 128, csl])

                    if ones_t is None:
                        ones_t = const_pool.tile([128, 1], BF16)
                        nc.sync.dma_start(ones_t[:], ones_d[:, :])
                        maskf_t = const_pool.tile([128, BL * NPB], F32)
                        nc.sync.dma_start(maskf_t[:], mk_d[:, :])

                    num_t = tmp_pool.tile([128, HHX], BF16, tag="num")
                    nc.vector.tensor_mul(num_t[:], u_t[:], m_t[:])
                    uu_t = tmp_pool.tile([128, HHX], BF16, tag="uu")
                    nc.scalar.square(uu_t[:], u_t[:])
                    mm_t = tmp_pool.tile([128, HHX], BF16, tag="mm")
                    # spread m*m across the three elementwise-capable
                    # engines so none exceeds the DMA roofline
                    r = k % 3
                    if r == 0:
                        nc.scalar.square(mm_t[:], m_t[:])
                    elif r == 1:
                        nc.vector.tensor_mul(mm_t[:], m_t[:], m_t[:])
                    elif _F_GPSIMD_MM:
                        nc.gpsimd.tensor_mul(mm_t[:], m_t[:], m_t[:])
                    else:
                        nc.scalar.square(mm_t[:], m_t[:])
                    k += 1

                    # start=True zeroes the tile's whole 2KB PSUM bank, so
                    # only the first matmul into P starts the group; the
                    # last one (ch1/h1/s2/pb15) closes it.
                    for s, t in enumerate((num_t, uu_t, mm_t)):
                        if _F_PSUM_ACC:
                            base = s * NPB + h * HPB
                        else:
                            base = ch * 3 * NPB + s * NPB + h * HPB
                        for pb in range(HPB):
                            if _F_PSUM_ACC:
                                first = (
                                    ch == 0 and h == 0 and s == 0 and pb == 0
                                )
                                last = (
                                    ch == NCHUNK - 1
                                    and h == 1
                                    and s == 2
                                    and pb == HPB - 1
                                )
                            else:
                                first = last = True
                            nc.tensor.matmul(
                                P[:, base + pb : base + pb + 1],
                                t[:, pb * 128 : (pb + 1) * 128],
                                ones_t[:, :],
                                start=first,
                                stop=last,
                            )

            # epilogue for batch b (position-major [128, NPB] psum slices)
            if _F_PSUM_ACC:
                num_P = P[:, 0:NPB]
                uu_P = P[:, NPB : 2 * NPB]
                mm_P = P[:, 2 * NPB : 3 * NPB]
                # DVE has a single PSUM read port: stage mm in SBUF via ACT
                mm_s = ep_pool.tile([128, NPB], F32, tag="mm_s")
                nc.scalar.copy(mm_s[:], mm_P)
                d2 = ep_pool.tile([128, NPB], F32, tag="d2")
                nc.vector.tensor_mul(d2[:], uu_P, mm_s[:])
            else:

                def psl(ch, s):
                    c0 = ch * 3 * NPB + s * NPB
                    return P[:, c0 : c0 + NPB]

                n0 = ep_pool.tile([128, NPB], F32, tag="n0")
                nc.scalar.copy(n0[:], psl(0, 0))
                u0 = ep_pool.tile([128, NPB], F32, tag="u0")
                nc.scalar.copy(u0[:], psl(0, 1))
                m0 = ep_pool.tile([128, NPB], F32, tag="m0")
                nc.scalar.copy(m0[:], psl(0, 2))
                num_s = ep_pool.tile([128, NPB], F32, tag="num_s")
                nc.vector.tensor_add(num_s[:], n0[:], psl(1, 0))
                uu_s = ep_pool.tile([128, NPB], F32, tag="uu_s")
                nc.vector.tensor_add(uu_s[:], u0[:], psl(1, 1))
                mm_s = ep_pool.tile([128, NPB], F32, tag="mm_s")
                nc.vector.tensor_add(mm_s[:], m0[:], psl(1, 2))
                num_P = num_s[:]
                d2 = ep_pool.tile([128, NPB], F32, tag="d2")
                nc.vector.tensor_mul(d2[:], uu_s[:], mm_s[:])
            rcp = ep_pool.tile([128, NPB], F32, tag="rcp")
            nc.vector.reciprocal(rcp[:], d2[:])
            rs = ep_pool.tile([128, NPB], F32, tag="rs")
            nc.scalar.sqrt(rs[:], rcp[:])
            rsm = ep_pool.tile([128, NPB], F32, tag="rsm")
            nc.vector.tensor_mul(
                rsm[:], rs[:], maskf_t[:, b * NPB : (b + 1) * NPB]
            )
            if _F_TTR:
                simmask = ep_pool.tile([128, NPB], F32, tag="simmask")
                nc.vector.tensor_tensor_reduce(
                    out=simmask[:],
                    in0=num_P,
                    in1=rsm[:],
                    scale=1.0,
                    scalar=0.0,
                    op0=mybir.AluOpType.mult,
                    op1=mybir.AluOpType.add,
                    accum_out=acc_t[:, b : b + 1],
                )
            else:
                simmask = ep_pool.tile([128, NPB], F32, tag="simmask")
                nc.vector.tensor_mul(simmask[:], num_P, rsm[:])
                nc.vector.tensor_reduce(
                    acc_t[:, b : b + 1],
                    simmask[:],
                    axis=mybir.AxisListType.X,
                    op=mybir.AluOpType.add,
                )

        nc.sync.dma_start(out_d[:, :], acc_t[:])

    nc.compile()
    return nc


def get_nc():
    global _CACHED_NC
    if _CACHED_NC is None:
        _CACHED_NC = build_nc()
    return _CACHED_NC


def make_in_maps(unmasked, masked, latent_mask):
    ones = np.ones((128, 1), dtype=ml_dtypes.bfloat16)
    in_maps = []
    for i in range(NCORES):
        sl = slice(i * BL, (i + 1) * BL)
        u = np.ascontiguousarray(unmasked[sl]).reshape(ROWS, HWX)
        m = np.ascontiguousarray(masked[sl]).reshape(ROWS, HWX)
        mk = (
            latent_mask[sl]
            .reshape(128, 128)
            .T.astype(np.float32)
        )
        in_maps.append(
            {
                "u": u,
                "m": m,
                "maskf": np.ascontiguousarray(mk),
                "ones": ones,
            }
        )
    return in_maps


def _finalize(results, latent_mask):
    num = 0.0
    for res in results:
        num += np.asarray(res["out"], dtype=np.float64).sum()
    den = float((np.asarray(latent_mask) != 0).sum())
    return np.float32(num / den)


def kernel(unmasked_latent_tensors, masked_latent_tensors, latent_mask, **kw):
    nc = get_nc()
    lm = np.asarray(latent_mask)
    in_maps = make_in_maps(
        np.asarray(unmasked_latent_tensors, dtype=np.float32),
        np.asarray(masked_latent_tensors, dtype=np.float32),
        lm,
    )
    res = run_bass_kernel_spmd(nc, in_maps, list(range(NCORES)))
    return _finalize(res.results, lm)


def kernel_traced(unmasked_latent_tensors, masked_latent_tensors, latent_mask):
    """Like kernel() but with NTFF tracing; returns (value, BassKernelResults)."""
    nc = get_nc()
    lm = np.asarray(latent_mask)
    in_maps = make_in_maps(
        np.asarray(unmasked_latent_tensors, dtype=np.float32),
        np.asarray(masked_latent_tensors, dtype=np.float32),
        lm,
    )
    res = run_bass_kernel_spmd(nc, in_maps, list(range(NCORES)), trace=True)
    return _finalize(res.results, lm), res
